# revision 18
# baseline (speedup 1.0000x reference)
"""Trainium2 Bass kernel for nn_BidirectionalTrustModel.

Computes, for each of N=65536 independent observation sequences:
  1. A sequential scan over T=64 steps updating a per-sequence trust
     interval [low, high] for 2 capability dims (sens, proc).
  2. trust = prod over dims of (sum_j d_j * m_j / sum_j m_j) where
     m is the 10-bin interval mask and d_j = (1+exp(beta*(req-s_j)))^(-zeta^2).
     (The reference's 10x10 outer-product normalization factorizes exactly.)

Sharding: pure data-parallel over N across 8 NeuronCores (8192 seqs/core).

Device scan runs in a scaled fp16 domain. Caps are relabeled onto the
non-uniform half-integer grid CP = [1,3.5,5.5,7.5,10,12,14,15.5,17.5,19]
(bin k -> CP[k]); initial interval [0, 20]; B-neutral 40. CP is chosen so
that every fp32 comparison outcome of the reference (including the inexact
`high2 - 0.1` fixup value vs neighbouring grid/step constants, where steps
== caps bitwise because XLA lowers /10 to *0.1f) is reproduced by exact
half-integer arithmetic, with the fixup value always exactly `high - 2`.
All scan values are half-integers in [-2, 40] => exact in fp16, and fp16
tensor_tensor ops run at 2 elem/cycle on the DVE.

Per step, with state X = [lo | v], v = 40 - hi, and per-step planes
P = [B | 40-A], M = [A | 40-B] (A = cap on success else 0, B = cap on
failure else 40):
    q1 = (v <= 40-A)            # success-above guard (A > hi)
    q2 = (lo <= B)              # failure-below guard (B < lo)
    lo2 = max(min(lo, B), A*q1) # == min(max(lo, A*q1), B)  (A*q1 <= B)
    v2  = max(min(v, 40-A), (40-B)*q2)
    eq  = (lo2 + v2 >= 40)      # collapse: lo2 == hi2
    lo' = lo2 - 2*eq            # fixup (CP makes hi-2 the exact fp32 image)
"""

import numpy as np

BINS = 10
T = 64
N_TOTAL = 65536
N_CORES = 8
P = 128                  # SBUF partitions
NC = N_TOTAL // N_CORES  # 8192 sequences per core
K = NC // P              # 64 sequence-columns per dim
F4 = T * K               # 4096 cols for [T, NC] planes laid out [P, T*K]
W = 2 * K                # 128: both dims side by side
XW = 2 * W               # 256: [lo | v] halves

_F32 = np.float32
# reference step constants in fp32 (for the d-weights, unscaled)
STEPS = ((np.arange(BINS, dtype=np.float32) + _F32(0.5)) * _F32(0.1)).astype(np.float32)
# scaled non-uniform grid (see module docstring)
CP = np.array([1, 3.5, 5.5, 7.5, 10, 12, 14, 15.5, 17.5, 19], np.float32)

# t-chunk sizes for DMA + precompute pipelining (small first chunks let the
# DVE scan start early)
CHUNK_STEPS = [2, 2, 4, 8, 8, 8, 8, 8, 8, 8]
assert sum(CHUNK_STEPS) == T

_NC_CACHE = {}


def _build_nc():
    import concourse.bass as bass
    import concourse.mybir as mybir
    import concourse.tile as tile
    from concourse.tile import ScopedClock

    dt = mybir.dt
    Alu = mybir.AluOpType
    Act = mybir.ActivationFunctionType

    class PatchedTileContext(tile.TileContext):
        """This walrus build only lowers ONE sem wait per SP Drain; split the
        tail drain's waits across extra drain instructions."""
        MAX_WAITS = 1

        def _drain_and_barrier(self, tick_clock, wait_clock):
            nc = self.nc
            drain_inst = nc.sync.drain()
            wait_clock.add_sem_waits(
                drain_inst.ins, ScopedClock({None: tick_clock.global_clock})
            )
            si = drain_inst.ins.sync_info
            kmax = self.MAX_WAITS
            if si is not None and si.on_wait and len(si.on_wait) > kmax:
                waits = list(si.on_wait)
                drain_inst.ins.sync_info = mybir.SyncInfo(
                    on_wait=waits[:kmax], on_update=list(si.on_update)
                )
                rest = waits[kmax:]
                for i in range(0, len(rest), kmax):
                    extra = nc.sync.drain()
                    extra.ins.sync_info = mybir.SyncInfo(
                        on_wait=rest[i : i + kmax], on_update=[]
                    )
            nc.all_engine_barrier()
            assert self.sems is not None
            popped = nc._tile_sem_poison_stack.pop()
            assert popped is self._sem_poison
            nc.clear_and_free_semaphores(list(self.sems.allocated().values()))
            nc.all_engine_barrier()

    def _split_sync_waits(nc):
        """This walrus build lowers at most ONE sync wait per instruction.
        Move extra waits onto same-engine NoOps inserted just before."""
        n_split = 0
        for f in nc.m.functions:
            for bb in f.blocks:
                il = bb.instructions
                new = []
                for ins in il:
                    si = ins.sync_info
                    if si is not None and si.on_wait and len(si.on_wait) > 1:
                        waits = list(si.on_wait)
                        for w in waits[:-1]:
                            nop = mybir.InstNoOp(name=f"I-wsplit-{nc.next_id()}")
                            nop.engine = ins.engine
                            nop.sync_info = mybir.SyncInfo(on_wait=[w], on_update=[])
                            nc.register_instruction(nop, overwrite=True)
                            new.append(nop)
                            n_split += 1
                        ins.sync_info = mybir.SyncInfo(
                            on_wait=[waits[-1]], on_update=list(si.on_update)
                        )
                    new.append(ins)
                il[:] = new
        return n_split

    nc = bass.Bass(target_bir_lowering=False, trn_type="TRN2")

    f32, f16 = dt.float32, dt.float16
    blk_d = nc.declare_dram_parameter("blk", [P, 4 * F4], f16, isOutput=False)
    sjc_d = nc.declare_dram_parameter("sjc", [P, K * BINS], f16, isOutput=False)
    bt_d = nc.declare_dram_parameter("bt", [P, K * BINS], f32, isOutput=False)
    reqb_s_d = nc.declare_dram_parameter("reqb_s", [P, K * BINS], f32, isOutput=False)
    reqb_p_d = nc.declare_dram_parameter("reqb_p", [P, K * BINS], f32, isOutput=False)
    bz_d = nc.declare_dram_parameter("bz", [P, 4], f32, isOutput=False)
    out_d = nc.declare_dram_parameter("trust", [P, K], f32, isOutput=True)

    NCHUNK = len(CHUNK_STEPS)
    CHUNK_T0 = [sum(CHUNK_STEPS[:c]) for c in range(NCHUNK)]

    with PatchedTileContext(nc) as tc:
        with tc.tile_pool(name="planes", bufs=1) as planes:
            # Per-chunk P/M plane tiles (separate tiles keep Tile's dependency
            # tracking precise: the scan of chunk c only waits on chunk c's
            # precompute). Within a chunk, cols = t_local*256 + half*128 +
            # dim*64 + k;  half0 of Pt is B, half1 is 40-A; half0 of Mt is A,
            # half1 is 40-B.
            P_chunks = [planes.tile([P, CHUNK_STEPS[c] * XW], f16,
                                    tag=f"P{c}", name=f"Pch{c}")
                        for c in range(NCHUNK)]
            M_chunks = [planes.tile([P, CHUNK_STEPS[c] * XW], f16,
                                    tag=f"M{c}", name=f"Mch{c}")
                        for c in range(NCHUNK)]

            # Keep every pool open for the whole kernel (closing one lets the
            # allocator reuse its SBUF range and Tile then serializes across
            # the released-zone hazard, destroying the pipeline).
            with tc.tile_pool(name="stage", bufs=3) as stage, \
                 tc.tile_pool(name="state", bufs=1) as state_pool, \
                 tc.tile_pool(name="scantmp", bufs=6) as stp, \
                 tc.tile_pool(name="final", bufs=1) as fin:

                # constants
                k40 = state_pool.tile([P, W], f16, tag="k40")
                nc.vector.memset(k40[:], 40.0)

                CHMAX = max(CHUNK_STEPS) * K
                for c in range(NCHUNK):
                    TCH = CHUNK_STEPS[c]
                    CH = TCH * K
                    st4 = stage.tile([P, 4 * CHMAX], f16, tag="st4", name=f"st4_{c}")
                    cap_s = st4[:, 0 * CH : 1 * CH]
                    cap_p = st4[:, 1 * CH : 2 * CH]
                    p0 = st4[:, 2 * CH : 3 * CH]
                    p1 = st4[:, 3 * CH : 4 * CH]
                    t2 = stage.tile([P, CHMAX], f16, tag="t2", name=f"t2_{c}")[:, :CH]
                    tb = stage.tile([P, CHMAX], f16, tag="tb", name=f"tb_{c}")[:, :CH]
                    o0 = 4 * CHUNK_T0[c] * K
                    nc.sync.dma_start(st4[:, : 4 * CH], blk_d[:, o0 : o0 + 4 * CH])

                    def tk(ap):  # [P, TCH, K] view of a staging plane
                        return ap[:].rearrange("p (t k) -> p t k", k=K)

                    def blk(ch, half, dim):  # [P, TCH, K] view into P/M chunk
                        v = ch[:].rearrange("p (t w) -> p t w", w=XW)
                        o = half * W + dim * K
                        return v[:, :, o : o + K]

                    Pt, Mt = P_chunks[c], M_chunks[c]
                    if c == 0:
                        # chunk 0 on the DVE itself: shorter lead-in than
                        # GpSimd/ACT (scan consumes it immediately)
                        e = nc.vector
                        e.scalar_tensor_tensor(t2[:], p0[:], -40.0, k40[:, :CH],
                                               Alu.mult, Alu.add)
                        for dim, cap in ((0, cap_s), (1, cap_p)):
                            e.tensor_tensor(blk(Mt, 0, dim), tk(cap), tk(p1), Alu.mult)
                            e.tensor_tensor(tb[:], cap[:], p0[:], Alu.mult)
                            e.tensor_tensor(blk(Pt, 0, dim), tk(tb), tk(t2), Alu.add)
                            e.scalar_tensor_tensor(blk(Pt, 1, dim),
                                                   blk(Mt, 0, dim), -1.0,
                                                   tk(k40[:, :CH]), Alu.mult, Alu.add)
                            e.scalar_tensor_tensor(blk(Mt, 1, dim),
                                                   blk(Pt, 0, dim), -1.0,
                                                   tk(k40[:, :CH]), Alu.mult, Alu.add)
                    else:
                        g = nc.gpsimd
                        # t2 = 40 - 40*p0 on ACT; A/B products on GpSimd;
                        # 40-x derivations on ACT
                        nc.scalar.activation(t2[:], p0[:], Act.Copy,
                                             bias=40.0, scale=-40.0)
                        for dim, cap in ((0, cap_s), (1, cap_p)):
                            g.tensor_tensor(blk(Mt, 0, dim), tk(cap), tk(p1), Alu.mult)
                            g.tensor_tensor(tb[:], cap[:], p0[:], Alu.mult)
                            g.tensor_tensor(blk(Pt, 0, dim), tk(tb), tk(t2), Alu.add)
                            nc.scalar.activation(blk(Pt, 1, dim), blk(Mt, 0, dim),
                                                 Act.Copy, bias=40.0, scale=-1.0)
                            nc.scalar.activation(blk(Mt, 1, dim), blk(Pt, 0, dim),
                                                 Act.Copy, bias=40.0, scale=-1.0)

                # ---- d-weights (independent of the scan; emitted first so
                # the ACT engine computes them early) ----
                KB = K * BINS  # 640
                bt = fin.tile([P, KB], f32, tag="bt")
                reqb_s = fin.tile([P, KB], f32, tag="reqb_s")
                reqb_p = fin.tile([P, KB], f32, tag="reqb_p")
                bz = fin.tile([P, 4], f32, tag="bz")
                sjc = fin.tile([P, KB], f16, tag="sjc")
                sj40 = fin.tile([P, KB], f16, tag="sj40")
                nc.sync.dma_start(bt[:], bt_d[:, :])
                nc.sync.dma_start(reqb_s[:], reqb_s_d[:, :])
                nc.sync.dma_start(reqb_p[:], reqb_p_d[:, :])
                nc.sync.dma_start(bz[:], bz_d[:, :])
                nc.sync.dma_start(sjc[:], sjc_d[:, :])
                nc.scalar.activation(sj40[:], sjc[:], Act.Copy,
                                     bias=40.0, scale=-1.0)

                nzz = fin.tile([P, 2], f32, tag="nzz")
                nc.gpsimd.tensor_tensor(nzz[:], bz[:, 2:4], bz[:, 2:4], Alu.mult)
                nc.gpsimd.tensor_scalar(nzz[:], nzz[:], -1.0, None, Alu.mult)

                d_tiles = []
                for dim, reqb in ((0, reqb_s), (1, reqb_p)):
                    t1 = fin.tile([P, KB], f32, tag=f"t1_{dim}")
                    sp = fin.tile([P, KB], f32, tag=f"sp_{dim}")
                    dti = fin.tile([P, KB], f32, tag=f"d_{dim}")
                    # d = exp(-zeta^2 * ln(1 + exp(beta * (req - s))))
                    nc.gpsimd.tensor_tensor(t1[:], reqb[:], bt[:], Alu.subtract)
                    nc.scalar.activation(sp[:], t1[:], Act.Exp,
                                         scale=bz[:, dim : dim + 1])
                    nc.gpsimd.tensor_scalar(t1[:], sp[:], 1.0, None, Alu.add)
                    nc.scalar.activation(sp[:], t1[:], Act.Ln)
                    nc.scalar.activation(dti[:], sp[:], Act.Exp,
                                         scale=nzz[:, dim : dim + 1])
                    d_tiles.append(dti)

                # ---- the scan ----
                # State lives in per-step-rotated SEPARATE lo / v tiles: Tile
                # tracks dependencies per-tile, so every op here is a single
                # full-tile write with true-only dependencies (no WAW/WAR
                # serialization, no false cross-half waits).
                # The collapse test lo2==hi2 fires iff the step's event value
                # hits the matching boundary exactly: eq = [A==hi] | [B==lo]
                # = [v==40-A] | [lo==B] (A-neutral 0 < 1 <= hi and B-neutral
                # 40 > 19 >= lo can't fire; succ/fail exclusive => OR == add).
                # This keeps the fixup test OFF the serial chain.
                X_a = state_pool.tile([P, XW], f16, tag="Xa")
                X_b = state_pool.tile([P, XW], f16, tag="Xb")
                nc.vector.memset(X_a[:, 0:W], 0.0)
                nc.vector.memset(X_a[:, W:XW], 20.0)

                t_to_chunk = []
                for c in range(NCHUNK):
                    t_to_chunk += [(c, i) for i in range(CHUNK_STEPS[c])]

                Xp = X_a
                for t in range(T):
                    tc_i, tl = t_to_chunk[t]
                    Pv = P_chunks[tc_i][:, tl * XW : (tl + 1) * XW]
                    Mv = M_chunks[tc_i][:, tl * XW : (tl + 1) * XW]
                    Xc = X_b if (t % 2 == 0) else X_a
                    q = stp.tile([P, XW], f16, tag="q")
                    Wt = stp.tile([P, XW], f16, tag="Wt")
                    Qt = stp.tile([P, XW], f16, tag="Qt")
                    eh = stp.tile([P, XW], f16, tag="eh")
                    eq = stp.tile([P, W], f16, tag="eq")

                    nc.vector.tensor_tensor(Wt[:], Xp[:], Pv, Alu.min)
                    nc.vector.tensor_tensor(q[:, 0:W], Xp[:, W:XW],
                                            Pv[:, W:XW], Alu.is_le)
                    nc.vector.tensor_tensor(q[:, W:XW], Xp[:, 0:W],
                                            Pv[:, 0:W], Alu.is_le)
                    nc.vector.tensor_tensor(eh[:], Xp[:], Pv, Alu.is_equal)
                    nc.vector.tensor_tensor(Qt[:], Mv, q[:], Alu.mult)
                    nc.vector.tensor_tensor(eq[:], eh[:, 0:W], eh[:, W:XW],
                                            Alu.add)
                    nc.vector.tensor_tensor(Xc[:], Wt[:], Qt[:], Alu.max)
                    nc.vector.scalar_tensor_tensor(Xc[:, 0:W], eq[:], -2.0,
                                                   Xc[:, 0:W], Alu.mult, Alu.add)
                    Xp = Xc

                # ---- final phase: masks (scaled f16, exact), then trust ----
                lo = Xp[:, 0:W]   # [lo_s | lo_p]
                vv = Xp[:, W:XW]  # [v_s | v_p]
                sjc3 = sjc[:].rearrange("p (k j) -> p k j", j=BINS)
                sj403 = sj40[:].rearrange("p (k j) -> p k j", j=BINS)
                UC = []
                for dim in (0, 1):
                    lob = lo[:, dim * K:(dim + 1) * K].unsqueeze(2).broadcast_to((P, K, BINS))
                    vb = vv[:, dim * K:(dim + 1) * K].unsqueeze(2).broadcast_to((P, K, BINS))
                    m1 = fin.tile([P, KB], f16, tag=f"m1_{dim}")
                    m2 = fin.tile([P, KB], f16, tag=f"m2_{dim}")
                    mm = fin.tile([P, KB], f16, tag=f"m_{dim}")
                    mf = fin.tile([P, KB], f32, tag=f"mf_{dim}")
                    dm = fin.tile([P, KB], f32, tag=f"dm_{dim}")
                    # m = (s' >= lo) & (s' <= hi)  <=>  (s' >= lo) & (40-s' >= v)
                    # mask compares on DVE (Pool rejects f16/broadcast TT);
                    # dim1's f32 multiply+reductions go to GpSimd in parallel.
                    nc.vector.tensor_tensor(m1[:].rearrange("p (k j) -> p k j", j=BINS),
                                            sjc3, lob, Alu.is_ge)
                    nc.vector.tensor_tensor(m2[:].rearrange("p (k j) -> p k j", j=BINS),
                                            sj403, vb, Alu.is_ge)
                    nc.vector.tensor_tensor(mm[:], m1[:], m2[:], Alu.mult)
                    nc.scalar.activation(mf[:], mm[:], Act.Copy)  # f16 -> f32
                    reng = nc.vector if dim == 0 else nc.gpsimd
                    reng.tensor_tensor(dm[:], d_tiles[dim][:], mf[:], Alu.mult)

                    U = fin.tile([P, K], f32, tag=f"U_{dim}")
                    C = fin.tile([P, K], f32, tag=f"C_{dim}")
                    nc.vector.tensor_reduce(
                        U[:], dm[:].rearrange("p (k j) -> p k j", j=BINS),
                        mybir.AxisListType.X, Alu.add)
                    nc.vector.tensor_reduce(
                        C[:], mf[:].rearrange("p (k j) -> p k j", j=BINS),
                        mybir.AxisListType.X, Alu.add)
                    UC.append((U, C))

                # trust = (U0*U1) / (C0*C1)
                uu = fin.tile([P, K], f32, tag="uu")
                cc = fin.tile([P, K], f32, tag="cc")
                rr = fin.tile([P, K], f32, tag="rr")
                tr = fin.tile([P, K], f32, tag="tr")
                nc.vector.tensor_tensor(uu[:], UC[0][0][:], UC[1][0][:], Alu.mult)
                nc.vector.tensor_tensor(cc[:], UC[0][1][:], UC[1][1][:], Alu.mult)
                nc.vector.reciprocal(rr[:], cc[:])
                nc.vector.tensor_tensor(tr[:], uu[:], rr[:], Alu.mult)
                nc.sync.dma_start(out_d[:, :], tr[:])

    _split_sync_waits(nc)
    return nc


def _get_nc():
    if "nc" not in _NC_CACHE:
        _NC_CACHE["nc"] = _build_nc()
    return _NC_CACHE["nc"]


def _marshal_core(inputs, c):
    """Build the per-core input map (slicing/layout + grid relabeling)."""
    n0, n1 = c * NC, (c + 1) * NC

    def lay(x):  # [T, NC] -> [P, T*K] with col = t*K + k, seq n = p*K + k
        return np.ascontiguousarray(
            x.reshape(T, P, K).transpose(1, 0, 2).reshape(P, F4))

    def enc(x):  # fp32 grid caps -> scaled half-integer grid, f16
        k = np.round(np.asarray(x, np.float32) * 10 - 0.5).astype(np.int64)
        return CP[k].astype(np.float16)

    cap_s = lay(enc(inputs["obs_task_sens_cap_seq"][:, n0:n1]))
    cap_p = lay(enc(inputs["obs_task_proc_cap_seq"][:, n0:n1]))
    perf = np.asarray(inputs["inptasksperf"][:, n0:n1, :])
    p0 = lay(np.ascontiguousarray(perf[:, :, 0]).astype(np.float16))
    p1 = lay(np.ascontiguousarray(perf[:, :, 1]).astype(np.float16))
    t0s = np.cumsum([0] + CHUNK_STEPS[:-1]).tolist()
    blk = np.concatenate(
        [np.concatenate([a[:, t0 * K : (t0 + tc) * K]
                         for a in (cap_s, cap_p, p0, p1)], axis=1)
         for t0, tc in zip(t0s, CHUNK_STEPS)], axis=1)
    blk = np.ascontiguousarray(blk)

    def layreq(x):  # [NC] -> [P, K*BINS] broadcast each seq over 10 bins
        r = x.reshape(P, K, 1)
        return np.ascontiguousarray(np.broadcast_to(r, (P, K, BINS)).reshape(P, K * BINS))

    req_s = layreq(np.asarray(inputs["pred_task_sens_cap"][n0:n1, 0], dtype=np.float32))
    req_p = layreq(np.asarray(inputs["pred_task_proc_cap"][n0:n1, 0], dtype=np.float32))
    bt = np.ascontiguousarray(np.broadcast_to(np.tile(STEPS, K), (P, K * BINS))).astype(np.float32)
    sjc = np.ascontiguousarray(np.broadcast_to(
        np.tile(CP.astype(np.float16), K), (P, K * BINS)))
    betas = np.asarray(inputs["betas"], dtype=np.float32)
    zetas = np.asarray(inputs["zetas"], dtype=np.float32)
    bz = np.ascontiguousarray(
        np.broadcast_to(np.concatenate([betas, zetas]).astype(np.float32), (P, 4)))
    return {
        "blk": blk, "sjc": sjc,
        "bt": bt, "reqb_s": req_s, "reqb_p": req_p, "bz": bz,
    }


def kernel(**inputs) -> np.ndarray:
    from concourse.bass_utils import run_bass_kernel_spmd

    nc = _get_nc()
    in_maps = [_marshal_core(inputs, c) for c in range(N_CORES)]
    res = run_bass_kernel_spmd(nc, in_maps, core_ids=list(range(N_CORES)))
    out = np.empty((N_TOTAL, 1), dtype=np.float32)
    for c in range(N_CORES):
        out[c * NC : (c + 1) * NC, 0] = res.results[c]["trust"].reshape(NC)
    return out
